# revision 36
# baseline (speedup 1.0000x reference)
"""Trainium2 Bass kernel for CrossAttention.

Problem shape (hardcoded):
  latent  [8, 4096, 512], context [8, 77, 768]
  wq [512,512], wk/wv [768,512], wo [512,512], biases [512]
  out = softmax((latent@wq+bq)(context@wk+bk)^T / 8) @ (context@wv+bv) @ wo + bo

Sharding: data-parallel over batch — core b handles batch element b.

Perf design (v9), from trace analysis of earlier versions:
  - v1 was LDWEIGHTS-bound (~2400 small matmuls with 64/77-col weights ->
    no fast-weight-load, PE weight port saturated at ~110ns/MM).
  - Restructured to N=512 streaming matmuls with 128-col bf16 weights:
    steady-state MM pitch hits the theoretical 216ns.
  - Software pipelining for the in-order PE queue: stage A (transposes +
    Q-proj) of block b+1 is emitted inside stage B of block b, and the
    out-projection of block b is DEFERRED into iteration b+1 so the PE
    chews on it while block b+1's softmax/PV chain waits on ACT/DVE.
  - Inputs arrive pre-cast to bf16 (host-side dtype choice, numerically
    identical to an on-device cast); x rides the fast SWDGE queue, stores
    are spread across the sync/scalar/gpsimd DMA queues.

Dataflow per core (one batch element), bf16 matmul operands:
  prep: cT = ctx^T; kT2 [128,8,77] head-parity layout (head h in rows
        (h%2)*64..+64, other half zero -> one matmul against the full
        128-row head-pair qT tile selects head h); v_hm [128,8,64] V per
        head kv-major; ones_kv (softmax-sum lhsT); bo_bc [128,512].
  per 512-row block:
    xT = x^T (16 PE transposes), qT = wq^T xT + bq  [128,4,512] bf16
    sT_h = kT2_h^T qT_pair -> [77,512] psum; eT_h = exp(sT_h) (ACT)
    pv/sums per head PAIR stacked [even head rows 0:64 | odd 64:128] ->
        full-tile reciprocal_approx_fast + multiply on DVE (all
        base-partition-0: custom DVE ops mishandle base-64 inputs)
    out = oT^T wo (+bo via DVE add) -> DMA (deferred one iteration)
"""

import os
import sys
from contextlib import ExitStack

import numpy as np

for _p in ("/opt/trn_rl_repo",):
    if _p not in sys.path and os.path.isdir(_p):
        sys.path.insert(0, _p)

import concourse.bass as bass  # noqa: E402
import concourse.tile as tile  # noqa: E402
from concourse import bacc, mybir  # noqa: E402
from concourse.bass_utils import run_bass_kernel_spmd  # noqa: E402
from concourse.masks import make_identity  # noqa: E402

N_CORES = 8
MM_DT = "bf16"  # informational (test.py prints it)
SQ, D, DC, SKV, H, DH = 4096, 512, 768, 77, 8, 64
F32 = mybir.dt.float32
BF16 = mybir.dt.bfloat16
AF = mybir.ActivationFunctionType
ALU = mybir.AluOpType

N_BLOCKS = SQ // 512  # 8 blocks of 512 query rows


def build_nc():
    nc = bacc.Bacc("TRN2", target_bir_lowering=False, debug=False)

    lat = nc.dram_tensor("latent", [SQ, D], BF16, kind="ExternalInput").ap()
    ctx_d = nc.dram_tensor("context", [SKV, DC], BF16, kind="ExternalInput").ap()
    wq = nc.dram_tensor("wq", [D, D], BF16, kind="ExternalInput").ap()
    bq = nc.dram_tensor("bq", [D], F32, kind="ExternalInput").ap()
    wk = nc.dram_tensor("wk", [DC, D], BF16, kind="ExternalInput").ap()
    bk = nc.dram_tensor("bk", [D], F32, kind="ExternalInput").ap()
    wv = nc.dram_tensor("wv", [DC, D], BF16, kind="ExternalInput").ap()
    bv = nc.dram_tensor("bv", [D], BF16, kind="ExternalInput").ap()
    wo = nc.dram_tensor("wo", [D, D], BF16, kind="ExternalInput").ap()
    bo = nc.dram_tensor("bo", [D], BF16, kind="ExternalInput").ap()
    out_d = nc.dram_tensor("out", [SQ, D], F32, kind="ExternalOutput").ap()

    with tile.TileContext(nc) as tc:
        with ExitStack() as stk:
            consts = stk.enter_context(tc.tile_pool(name="consts", bufs=1))
            xtp = stk.enter_context(tc.tile_pool(name="xt", bufs=3))
            qtp = stk.enter_context(tc.tile_pool(name="qt", bufs=2))
            otp = stk.enter_context(tc.tile_pool(name="ot", bufs=2))
            rsp = stk.enter_context(tc.tile_pool(name="rs", bufs=2))
            outp = stk.enter_context(tc.tile_pool(name="outp", bufs=3))
            # PSUM: 8 banks. trq: transposes + q-proj (2); spv: scores +
            # pv/sums (4); op: out-proj (2).
            trq = stk.enter_context(tc.tile_pool(name="trq", bufs=2, space="PSUM"))
            spv = stk.enter_context(tc.tile_pool(name="spv", bufs=4, space="PSUM"))
            op = stk.enter_context(tc.tile_pool(name="op", bufs=2, space="PSUM"))

            # ---------- constants the PE needs first (emitted before any
            # DMA so nothing queues behind SWDGE transfers) ----------
            ident = consts.tile([128, 128], BF16, name="ident")
            make_identity(nc, ident)
            e0 = consts.tile([128, 128], BF16, name="e0")
            nc.vector.memset(e0, 0.0)
            nc.vector.memset(e0[0:1, :], 1.0)
            eT_bufs = []
            for i in range(2):
                eT = consts.tile([128, H, 512], BF16, name=f"eT{i}")
                nc.vector.memset(eT[64:128, :, :], 0.0)
                eT_bufs.append(eT)

            # ---------- loads: x arrives TRANSPOSED via XBAR DMA-transpose,
            # ALL on the sync queue (single-queue-per-tile is required for
            # correctness, and sync carries nothing else). Weights ride
            # SWDGE; ctx/biases on scalar. ----------
            xT_tiles = {}

            def xT_load(bi):
                """xT [128, 4(dchunk), 512(rows)] direct from HBM."""
                if bi >= N_BLOCKS:
                    return
                xT_sb = xtp.tile([128, 4, 512], BF16, tag="xT", name="xT_sb")
                r0 = bi * 512
                for t in range(4):
                    nc.sync.dma_start_transpose(
                        xT_sb[:, t, :],
                        lat[r0 : r0 + 512, t * 128 : (t + 1) * 128],
                    )
                xT_tiles[bi] = xT_sb

            def loadw(eng, ap_in, shape, name):
                t = consts.tile(shape, BF16, name=name)
                eng.dma_start(t, ap_in)
                return t

            wq_sb = loadw(nc.gpsimd, wq.rearrange("(t p) d -> p t d", p=128),
                          [128, 4, D], "wq_sb")
            xT_load(0)
            ctx_sb = consts.tile([128, DC], BF16, name="ctx_sb")
            nc.vector.memset(ctx_sb, 0.0)
            nc.scalar.dma_start(ctx_sb[:SKV, :], ctx_d)
            # wk/wv in halves so the K/V prep matmuls start on partial data
            wk_r = wk.rearrange("(t p) d -> p t d", p=128)
            wk_sb = consts.tile([128, 6, D], BF16, name="wk_sb")
            nc.gpsimd.dma_start(wk_sb[:, 0:3, :], wk_r[:, 0:3, :])
            nc.gpsimd.dma_start(wk_sb[:, 3:6, :], wk_r[:, 3:6, :])
            wv_r = wv.rearrange("(t p) d -> p t d", p=128)
            wv_sb = consts.tile([128, 6, D], BF16, name="wv_sb")
            nc.gpsimd.dma_start(wv_sb[:, 0:3, :], wv_r[:, 0:3, :])
            nc.gpsimd.dma_start(wv_sb[:, 3:6, :], wv_r[:, 3:6, :])
            xT_load(1)
            wo_sb = loadw(nc.gpsimd, wo.rearrange("(t p) d -> p t d", p=128),
                          [128, 4, D], "wo_sb")
            xT_load(2)

            bq_sb = consts.tile([128, 4], F32, name="bq_sb")
            nc.sync.dma_start(bq_sb, bq.rearrange("(t p) -> p t", p=128))
            bk_sb = consts.tile([128, 4], F32, name="bk_sb")
            nc.sync.dma_start(bk_sb, bk.rearrange("(t p) -> p t", p=128))
            bk_s = consts.tile([128, 4], F32, name="bk_s")
            nc.vector.tensor_scalar_mul(bk_s, bk_sb, 0.125)

            bv_pad = consts.tile([128, D], BF16, name="bv_pad")
            nc.vector.memset(bv_pad, 0.0)
            nc.sync.dma_start(bv_pad[0:1, :], bv.rearrange("(o d) -> o d", o=1))
            bo_pad = consts.tile([128, D], BF16, name="bo_pad")
            nc.vector.memset(bo_pad, 0.0)
            nc.sync.dma_start(bo_pad[0:1, :], bo.rearrange("(o d) -> o d", o=1))

            # ---------- stage A pieces (emitted fine-grained) ----------
            qT_tiles = {}

            def stageA_q(bi, ms):
                """Q-proj m-chunks: qT[:, m, :] = wq_m^T xT + bq (ACT bias)."""
                if bi >= N_BLOCKS:
                    return
                if bi not in qT_tiles:
                    qT_tiles[bi] = qtp.tile([128, 4, 512], BF16, tag="qT", name="qT_sb")
                xT_sb, qT_sb = xT_tiles[bi], qT_tiles[bi]
                for m in ms:
                    q_ps = trq.tile([128, 512], F32, tag="trq", name="q_ps")
                    for k in range(4):
                        nc.tensor.matmul(
                            q_ps,
                            lhsT=wq_sb[:, k, m * 128 : (m + 1) * 128],
                            rhs=xT_sb[:, k, :],
                            start=(k == 0),
                            stop=(k == 3),
                        )
                    nc.scalar.activation(
                        qT_sb[:, m, :], q_ps, AF.Identity,
                        bias=bq_sb[:, m : m + 1],
                    )
                if ms[-1] == 3:
                    xT_tiles.pop(bi)

            # deferred out-projection: emitted one iteration later so the
            # PE fills the tail of each block's DVE normalize chain
            oT_pending = {}

            def emit_out(bi):
                if bi < 0:
                    return
                oT_sb = oT_pending.pop(bi)
                store_eng = [nc.scalar, nc.gpsimd, nc.gpsimd, nc.scalar]
                for r in range(4):
                    o_ps = op.tile([128, 512], F32, tag="op", name="o_ps")
                    for t in range(4):
                        nc.tensor.matmul(
                            o_ps,
                            lhsT=oT_sb[:, t, r * 128 : (r + 1) * 128],
                            rhs=wo_sb[:, t, :],
                            start=(t == 0),
                            stop=(t == 3),
                        )
                    out_sb = outp.tile([128, D], F32, tag="out", name="out_sb")
                    nc.vector.tensor_tensor(out_sb, o_ps, bo_bc, ALU.add)
                    rr = bi * 512 + r * 128
                    store_eng[r].dma_start(out_d[rr : rr + 128, :], out_sb)

            # ---------- block 0 stage A before K/V prep (HAM warmup; x0/wq
            # arrive before SWDGE finishes wk/wv) ----------
            stageA_q(0, [0, 1, 2, 3])

            # ---------- K/V prep ----------
            cT_sb = consts.tile([128, 6, SKV], BF16, name="cT_sb")
            for g in range(2):
                cT_ps = trq.tile([128, 3, 128], BF16, tag="trq", name="cT_ps")
                for t3 in range(3):
                    t = g * 3 + t3
                    nc.tensor.transpose(
                        cT_ps[:, t3, :], ctx_sb[:, t * 128 : (t + 1) * 128], ident
                    )
                nc.vector.tensor_copy(cT_sb[:, 3 * g : 3 * g + 3, :], cT_ps[:, :, :SKV])

            kT2 = consts.tile([128, H, SKV], BF16, name="kT2")
            nc.vector.memset(kT2, 0.0)
            for t in range(4):
                kT_ps = spv.tile([128, SKV], F32, tag="spv", name="kT_ps")
                for ct in range(6):
                    nc.tensor.matmul(
                        kT_ps,
                        lhsT=wk_sb[:, ct, t * 128 : (t + 1) * 128],
                        rhs=cT_sb[:, ct, :],
                        start=(ct == 0),
                        stop=(ct == 5),
                    )
                nc.scalar.activation(
                    kT2[0:64, 2 * t, :], kT_ps[0:64, :], AF.Identity,
                    bias=bk_s[0:64, t : t + 1], scale=0.125,
                )
                nc.scalar.activation(
                    kT2[64:128, 2 * t + 1, :], kT_ps[64:128, :], AF.Identity,
                    bias=bk_s[64:128, t : t + 1], scale=0.125,
                )

            v_ps = spv.tile([SKV, D], F32, tag="spv", name="v_ps")
            for ct in range(6):
                nc.tensor.matmul(
                    v_ps, lhsT=cT_sb[:, ct, :], rhs=wv_sb[:, ct, :],
                    start=(ct == 0), stop=False,
                )
            nc.tensor.matmul(
                v_ps, lhsT=e0[:, :SKV], rhs=bv_pad, start=False, stop=True
            )
            v_hm = consts.tile([128, H, 64], BF16, name="v_hm")
            nc.vector.memset(v_hm, 0.0)
            for h in range(H):
                nc.vector.tensor_copy(
                    v_hm[0:SKV, h, :], v_ps[:, h * 64 : (h + 1) * 64]
                )
            ones_kv = consts.tile([128, 64], BF16, name="ones_kv")
            nc.vector.memset(ones_kv, 0.0)
            nc.vector.memset(ones_kv[0:64, :], 1.0)
            nc.vector.memset(ones_kv[64:SKV, :], 1.0)

            # bo broadcast to all 128 partitions via one rank-1 matmul
            bo_ps = op.tile([128, 512], F32, tag="op", name="bo_ps")
            nc.tensor.matmul(bo_ps, lhsT=e0, rhs=bo_pad, start=True, stop=True)
            bo_bc = consts.tile([128, D], F32, name="bo_bc")
            nc.vector.tensor_copy(bo_bc, bo_ps)

            # ---------- main loop ----------
            for bi in range(N_BLOCKS):
                eT = eT_bufs[bi % 2]
                qT_sb = qT_tiles.pop(bi)

                xT_load(bi + 3)

                def sT(h):
                    s_ps = spv.tile([SKV, 512], F32, tag="spv", name="s_ps")
                    nc.tensor.matmul(
                        s_ps, lhsT=kT2[:, h, :], rhs=qT_sb[:, h // 2, :],
                        start=True, stop=True,
                    )
                    nc.scalar.activation(eT[0:SKV, h, :], s_ps, AF.Exp)

                # scores interleaved with block bi+1's Q-proj (its xT was
                # prefetched two blocks ago)
                sT(0); sT(1); sT(2); sT(3)
                stageA_q(bi + 1, [0, 1])
                sT(4); sT(5); sT(6); sT(7)
                stageA_q(bi + 1, [2, 3])

                # PV + sums per head pair, then full-tile recip * mult
                oT_sb = otp.tile([128, 4, 512], BF16, tag="oT", name="oT_sb")
                for t in range(4):
                    pv_ps = spv.tile([128, 512], F32, tag="spv", name="pv_ps")
                    sm_ps = spv.tile([128, 512], F32, tag="spv", name="sm_ps")
                    for hh in range(2):
                        h = 2 * t + hh
                        o = hh * 64
                        nc.tensor.matmul(
                            pv_ps[o : o + 64, :], lhsT=v_hm[:, h, :],
                            rhs=eT[:, h, :], start=True, stop=True,
                        )
                        nc.tensor.matmul(
                            sm_ps[o : o + 64, :], lhsT=ones_kv,
                            rhs=eT[:, h, :], start=True, stop=True,
                        )
                    rs = rsp.tile([128, 512], F32, tag="rs", name="rs")
                    nc.vector.reciprocal_approx_fast(rs, sm_ps)
                    nc.vector.tensor_tensor(oT_sb[:, t, :], pv_ps, rs, ALU.mult)
                oT_pending[bi] = oT_sb

                # out-projection of the PREVIOUS block (its mults are long
                # done -> no PE stall)
                emit_out(bi - 1)

            emit_out(N_BLOCKS - 1)

    nc.compile()
    return nc


_BUILD_CACHE = {}


def _get_nc():
    if "nc" not in _BUILD_CACHE:
        _BUILD_CACHE["nc"] = build_nc()
    return _BUILD_CACHE["nc"]


def _in_maps(latent, context, wq, bq, wk, bk, wv, bv, wo, bo):
    import ml_dtypes

    bf = ml_dtypes.bfloat16
    f = lambda a: np.ascontiguousarray(np.asarray(a), dtype=np.float32)
    fb = lambda a: np.ascontiguousarray(
        np.asarray(a, dtype=np.float32).astype(bf)
    )
    shared = {
        "wq": fb(wq), "bq": f(bq), "wk": fb(wk), "bk": f(bk),
        "wv": fb(wv), "bv": fb(bv), "wo": fb(wo), "bo": fb(bo),
    }
    maps = []
    for b in range(N_CORES):
        m = dict(shared)
        m["latent"] = fb(latent[b])
        m["context"] = fb(context[b])
        maps.append(m)
    return maps


def run_on_hw(inputs, trace=False, **kw):
    nc = _get_nc()
    maps = _in_maps(**inputs)
    res = run_bass_kernel_spmd(nc, maps, list(range(N_CORES)), trace=trace, **kw)
    out = np.stack([res.results[b]["out"] for b in range(N_CORES)], axis=0)
    return out, res


def kernel(latent, context, wq, bq, wk, bk, wv, bv, wo, bo):
    out, _ = run_on_hw(dict(
        latent=latent, context=context, wq=wq, bq=bq, wk=wk, bk=bk,
        wv=wv, bv=bv, wo=wo, bo=bo,
    ))
    return out


# revision 39
# speedup vs baseline: 1.1189x; 1.1189x over previous
"""Trainium2 Bass kernel for CrossAttention.

Problem shape (hardcoded):
  latent  [8, 4096, 512], context [8, 77, 768]
  wq [512,512], wk/wv [768,512], wo [512,512], biases [512]
  out = softmax((latent@wq+bq)(context@wk+bk)^T / 8) @ (context@wv+bv) @ wo + bo

Sharding: data-parallel over batch — core b handles batch element b.

Perf design (v9), from trace analysis of earlier versions:
  - v1 was LDWEIGHTS-bound (~2400 small matmuls with 64/77-col weights ->
    no fast-weight-load, PE weight port saturated at ~110ns/MM).
  - Restructured to N=512 streaming matmuls with 128-col bf16 weights:
    steady-state MM pitch hits the theoretical 216ns.
  - Software pipelining for the in-order PE queue: stage A (transposes +
    Q-proj) of block b+1 is emitted inside stage B of block b, and the
    out-projection of block b is DEFERRED into iteration b+1 so the PE
    chews on it while block b+1's softmax/PV chain waits on ACT/DVE.
  - Inputs arrive pre-cast to bf16 (host-side dtype choice, numerically
    identical to an on-device cast); x rides the fast SWDGE queue, stores
    are spread across the sync/scalar/gpsimd DMA queues.

Dataflow per core (one batch element), bf16 matmul operands:
  prep: cT = ctx^T; kT2 [128,8,77] head-parity layout (head h in rows
        (h%2)*64..+64, other half zero -> one matmul against the full
        128-row head-pair qT tile selects head h); v_hm [128,8,64] V per
        head kv-major; ones_kv (softmax-sum lhsT); bo_bc [128,512].
  per 512-row block:
    xT = x^T (16 PE transposes), qT = wq^T xT + bq  [128,4,512] bf16
    sT_h = kT2_h^T qT_pair -> [77,512] psum; eT_h = exp(sT_h) (ACT)
    pv/sums per head PAIR stacked [even head rows 0:64 | odd 64:128] ->
        full-tile reciprocal_approx_fast + multiply on DVE (all
        base-partition-0: custom DVE ops mishandle base-64 inputs)
    out = oT^T wo (+bo via DVE add) -> DMA (deferred one iteration)
"""

import os
import sys
from contextlib import ExitStack

import numpy as np

for _p in ("/opt/trn_rl_repo",):
    if _p not in sys.path and os.path.isdir(_p):
        sys.path.insert(0, _p)

import concourse.bass as bass  # noqa: E402
import concourse.tile as tile  # noqa: E402
from concourse import bacc, mybir  # noqa: E402
from concourse.bass_utils import run_bass_kernel_spmd  # noqa: E402
from concourse.masks import make_identity  # noqa: E402

N_CORES = 8
MM_DT = "bf16"  # informational (test.py prints it)
SQ, D, DC, SKV, H, DH = 4096, 512, 768, 77, 8, 64
F32 = mybir.dt.float32
BF16 = mybir.dt.bfloat16
AF = mybir.ActivationFunctionType
ALU = mybir.AluOpType

N_BLOCKS = SQ // 512  # 8 blocks of 512 query rows


def build_nc():
    nc = bacc.Bacc("TRN2", target_bir_lowering=False, debug=False)

    lat = nc.dram_tensor("latent", [SQ, D], BF16, kind="ExternalInput").ap()
    ctx_d = nc.dram_tensor("context", [SKV, DC], BF16, kind="ExternalInput").ap()
    wq = nc.dram_tensor("wq", [D, D], BF16, kind="ExternalInput").ap()
    bq = nc.dram_tensor("bq", [D], F32, kind="ExternalInput").ap()
    wk = nc.dram_tensor("wk", [DC, D], BF16, kind="ExternalInput").ap()
    bk = nc.dram_tensor("bk", [D], F32, kind="ExternalInput").ap()
    wv = nc.dram_tensor("wv", [DC, D], BF16, kind="ExternalInput").ap()
    bv = nc.dram_tensor("bv", [D], BF16, kind="ExternalInput").ap()
    wo = nc.dram_tensor("wo", [D, D], BF16, kind="ExternalInput").ap()
    bo = nc.dram_tensor("bo", [D], BF16, kind="ExternalInput").ap()
    out_d = nc.dram_tensor("out", [SQ, D], F32, kind="ExternalOutput").ap()

    with tile.TileContext(nc) as tc:
        with ExitStack() as stk:
            consts = stk.enter_context(tc.tile_pool(name="consts", bufs=1))
            xpool = stk.enter_context(tc.tile_pool(name="x", bufs=3))
            xtp = stk.enter_context(tc.tile_pool(name="xt", bufs=2))
            qtp = stk.enter_context(tc.tile_pool(name="qt", bufs=2))
            otp = stk.enter_context(tc.tile_pool(name="ot", bufs=2))
            rsp = stk.enter_context(tc.tile_pool(name="rs", bufs=2))
            outp = stk.enter_context(tc.tile_pool(name="outp", bufs=3))
            # PSUM: 8 banks. trq: transposes + q-proj (2); spv: scores +
            # pv/sums (4); op: out-proj (2).
            trq = stk.enter_context(tc.tile_pool(name="trq", bufs=2, space="PSUM"))
            spv = stk.enter_context(tc.tile_pool(name="spv", bufs=4, space="PSUM"))
            op = stk.enter_context(tc.tile_pool(name="op", bufs=2, space="PSUM"))

            # ---------- constants the PE needs first (emitted before any
            # DMA so nothing queues behind SWDGE transfers) ----------
            ident = consts.tile([128, 128], BF16, name="ident")
            make_identity(nc, ident)
            e0 = consts.tile([128, 128], BF16, name="e0")
            nc.vector.memset(e0, 0.0)
            nc.vector.memset(e0[0:1, :], 1.0)
            eT_bufs = []
            for i in range(2):
                eT = consts.tile([128, H, 512], BF16, name=f"eT{i}")
                nc.vector.memset(eT[64:128, :, :], 0.0)
                eT_bufs.append(eT)

            # ---------- loads: x + bulk weights on SWDGE (fast), wq/ctx on
            # the HW queues so block 0 isn't gated on the SWDGE backlog ----
            x_tiles = {}

            def load_x(bi, split=False):
                if bi >= N_BLOCKS:
                    return
                t = xpool.tile([128, 4, D], BF16, tag="x", name=f"x{bi}")
                if split:
                    # per-g-chunk DMAs: the first transposes start as soon
                    # as g0 lands instead of waiting for the full 512 rows
                    for g in range(4):
                        r = bi * 512 + g * 128
                        nc.gpsimd.dma_start(t[:, g, :], lat[r : r + 128, :])
                else:
                    nc.gpsimd.dma_start(
                        t, lat[bi * 512 : (bi + 1) * 512, :].rearrange(
                            "(g p) d -> p g d", p=128
                        )
                    )
                x_tiles[bi] = t

            def loadw(eng, ap_in, shape, name):
                t = consts.tile(shape, BF16, name=name)
                eng.dma_start(t, ap_in)
                return t

            load_x(0, split=True)
            wq_sb = loadw(nc.sync, wq.rearrange("(t p) d -> p t d", p=128),
                          [128, 4, D], "wq_sb")
            ctx_sb = consts.tile([128, DC], BF16, name="ctx_sb")
            nc.vector.memset(ctx_sb, 0.0)
            nc.scalar.dma_start(ctx_sb[:SKV, :], ctx_d)
            # wk/wv in halves so the K/V prep matmuls start on partial data
            wk_r = wk.rearrange("(t p) d -> p t d", p=128)
            wk_sb = consts.tile([128, 6, D], BF16, name="wk_sb")
            nc.gpsimd.dma_start(wk_sb[:, 0:3, :], wk_r[:, 0:3, :])
            nc.gpsimd.dma_start(wk_sb[:, 3:6, :], wk_r[:, 3:6, :])
            wv_r = wv.rearrange("(t p) d -> p t d", p=128)
            wv_sb = consts.tile([128, 6, D], BF16, name="wv_sb")
            nc.gpsimd.dma_start(wv_sb[:, 0:3, :], wv_r[:, 0:3, :])
            nc.gpsimd.dma_start(wv_sb[:, 3:6, :], wv_r[:, 3:6, :])
            load_x(1)
            wo_sb = loadw(nc.gpsimd, wo.rearrange("(t p) d -> p t d", p=128),
                          [128, 4, D], "wo_sb")

            bq_sb = consts.tile([128, 4], F32, name="bq_sb")
            nc.sync.dma_start(bq_sb, bq.rearrange("(t p) -> p t", p=128))
            bk_sb = consts.tile([128, 4], F32, name="bk_sb")
            nc.sync.dma_start(bk_sb, bk.rearrange("(t p) -> p t", p=128))
            bk_s = consts.tile([128, 4], F32, name="bk_s")
            nc.vector.tensor_scalar_mul(bk_s, bk_sb, 0.125)

            bv_pad = consts.tile([128, D], BF16, name="bv_pad")
            nc.vector.memset(bv_pad, 0.0)
            nc.sync.dma_start(bv_pad[0:1, :], bv.rearrange("(o d) -> o d", o=1))
            bo_pad = consts.tile([128, D], BF16, name="bo_pad")
            nc.vector.memset(bo_pad, 0.0)
            nc.sync.dma_start(bo_pad[0:1, :], bo.rearrange("(o d) -> o d", o=1))

            # ---------- stage A pieces (emitted fine-grained) ----------
            xT_tiles, qT_tiles = {}, {}

            def stageA_tr(bi):
                """x^T via 16 PE transposes -> xT [128, 4(dchunk), 512]."""
                if bi >= N_BLOCKS:
                    return
                x_sb = x_tiles.pop(bi)
                xT_sb = xtp.tile([128, 4, 512], BF16, tag="xT", name="xT_sb")
                for g in range(4):
                    tr_ps = trq.tile([128, 4, 128], BF16, tag="trq", name="tr_ps")
                    for et in range(4):
                        nc.tensor.transpose(
                            tr_ps[:, et, :],
                            x_sb[:, g, et * 128 : (et + 1) * 128],
                            ident,
                        )
                    nc.vector.tensor_copy(
                        xT_sb[:, :, g * 128 : (g + 1) * 128], tr_ps
                    )
                xT_tiles[bi] = xT_sb

            q_ps_pend = {}

            def stageA_q_mm(bi, ms):
                """Q-proj matmuls only (evac emitted separately so block
                bi-1's exps stay ahead of these in the ACT FIFO)."""
                if bi >= N_BLOCKS:
                    return
                if bi not in qT_tiles:
                    qT_tiles[bi] = qtp.tile([128, 4, 512], BF16, tag="qT", name="qT_sb")
                xT_sb = xT_tiles[bi]
                for m in ms:
                    q_ps = trq.tile([128, 512], F32, tag="trq", name="q_ps")
                    for k in range(4):
                        nc.tensor.matmul(
                            q_ps,
                            lhsT=wq_sb[:, k, m * 128 : (m + 1) * 128],
                            rhs=xT_sb[:, k, :],
                            start=(k == 0),
                            stop=(k == 3),
                        )
                    q_ps_pend[(bi, m)] = q_ps
                if ms[-1] == 3:
                    xT_tiles.pop(bi)

            def stageA_q_evac(bi, ms):
                if bi >= N_BLOCKS:
                    return
                qT_sb = qT_tiles[bi]
                for m in ms:
                    nc.scalar.activation(
                        qT_sb[:, m, :], q_ps_pend.pop((bi, m)), AF.Identity,
                        bias=bq_sb[:, m : m + 1],
                    )

            def stageA_q(bi, ms):
                stageA_q_mm(bi, ms)
                stageA_q_evac(bi, ms)

            # deferred out-projection: emitted one iteration later so the
            # PE fills the tail of each block's DVE normalize chain
            oT_pending = {}

            def emit_out(bi):
                if bi < 0:
                    return
                oT_sb = oT_pending.pop(bi)
                store_eng = [nc.sync, nc.scalar, nc.gpsimd, nc.gpsimd]
                for r in range(4):
                    o_ps = op.tile([128, 512], F32, tag="op", name="o_ps")
                    for t in range(4):
                        nc.tensor.matmul(
                            o_ps,
                            lhsT=oT_sb[:, t, r * 128 : (r + 1) * 128],
                            rhs=wo_sb[:, t, :],
                            start=(t == 0),
                            stop=(t == 3),
                        )
                    out_sb = outp.tile([128, D], F32, tag="out", name="out_sb")
                    nc.vector.tensor_tensor(out_sb, o_ps, bo_bc, ALU.add)
                    rr = bi * 512 + r * 128
                    store_eng[r].dma_start(out_d[rr : rr + 128, :], out_sb)

            # ---------- block 0 stage A before K/V prep (HAM warmup; x0/wq
            # arrive before SWDGE finishes wk/wv) ----------
            stageA_tr(0)
            stageA_q(0, [0, 1, 2, 3])

            # ---------- K/V prep ----------
            cT_sb = consts.tile([128, 6, SKV], BF16, name="cT_sb")
            for g in range(2):
                cT_ps = trq.tile([128, 3, 128], BF16, tag="trq", name="cT_ps")
                for t3 in range(3):
                    t = g * 3 + t3
                    nc.tensor.transpose(
                        cT_ps[:, t3, :], ctx_sb[:, t * 128 : (t + 1) * 128], ident
                    )
                nc.vector.tensor_copy(cT_sb[:, 3 * g : 3 * g + 3, :], cT_ps[:, :, :SKV])

            kT2 = consts.tile([128, H, SKV], BF16, name="kT2")
            nc.vector.memset(kT2, 0.0)
            for t in range(4):
                kT_ps = spv.tile([128, SKV], F32, tag="spv", name="kT_ps")
                for ct in range(6):
                    nc.tensor.matmul(
                        kT_ps,
                        lhsT=wk_sb[:, ct, t * 128 : (t + 1) * 128],
                        rhs=cT_sb[:, ct, :],
                        start=(ct == 0),
                        stop=(ct == 5),
                    )
                nc.scalar.activation(
                    kT2[0:64, 2 * t, :], kT_ps[0:64, :], AF.Identity,
                    bias=bk_s[0:64, t : t + 1], scale=0.125,
                )
                nc.scalar.activation(
                    kT2[64:128, 2 * t + 1, :], kT_ps[64:128, :], AF.Identity,
                    bias=bk_s[64:128, t : t + 1], scale=0.125,
                )

            v_ps = spv.tile([SKV, D], F32, tag="spv", name="v_ps")
            for ct in range(6):
                nc.tensor.matmul(
                    v_ps, lhsT=cT_sb[:, ct, :], rhs=wv_sb[:, ct, :],
                    start=(ct == 0), stop=False,
                )
            nc.tensor.matmul(
                v_ps, lhsT=e0[:, :SKV], rhs=bv_pad, start=False, stop=True
            )
            v_hm = consts.tile([128, H, 64], BF16, name="v_hm")
            nc.vector.memset(v_hm, 0.0)
            for h in range(H):
                nc.vector.tensor_copy(
                    v_hm[0:SKV, h, :], v_ps[:, h * 64 : (h + 1) * 64]
                )
            ones_kv = consts.tile([128, 64], BF16, name="ones_kv")
            nc.vector.memset(ones_kv, 0.0)
            nc.vector.memset(ones_kv[0:64, :], 1.0)
            nc.vector.memset(ones_kv[64:SKV, :], 1.0)

            # bo broadcast to all 128 partitions via one rank-1 matmul
            bo_ps = op.tile([128, 512], F32, tag="op", name="bo_ps")
            nc.tensor.matmul(bo_ps, lhsT=e0, rhs=bo_pad, start=True, stop=True)
            bo_bc = consts.tile([128, D], F32, name="bo_bc")
            nc.vector.tensor_copy(bo_bc, bo_ps)

            # ---------- main loop ----------
            for bi in range(N_BLOCKS):
                eT = eT_bufs[bi % 2]
                qT_sb = qT_tiles.pop(bi)

                load_x(bi + 2)

                def sT(h):
                    s_ps = spv.tile([SKV, 512], F32, tag="spv", name="s_ps")
                    nc.tensor.matmul(
                        s_ps, lhsT=kT2[:, h, :], rhs=qT_sb[:, h // 2, :],
                        start=True, stop=True,
                    )
                    nc.scalar.activation(eT[0:SKV, h, :], s_ps, AF.Exp)

                # scores interleaved with block bi+1's transposes/Q-proj
                # matmuls; the Q-proj ACT evacs are deferred until after all
                # 8 exps so the ACT FIFO serves the softmax chain first
                sT(0); sT(1); sT(2); sT(3)
                stageA_tr(bi + 1)
                stageA_q_mm(bi + 1, [0, 1])
                sT(4); sT(5); sT(6); sT(7)

                # PV + sums per head pair, then full-tile recip * mult
                oT_sb = otp.tile([128, 4, 512], BF16, tag="oT", name="oT_sb")
                for t in range(4):
                    pv_ps = spv.tile([128, 512], F32, tag="spv", name="pv_ps")
                    sm_ps = spv.tile([128, 512], F32, tag="spv", name="sm_ps")
                    for hh in range(2):
                        h = 2 * t + hh
                        o = hh * 64
                        nc.tensor.matmul(
                            pv_ps[o : o + 64, :], lhsT=v_hm[:, h, :],
                            rhs=eT[:, h, :], start=True, stop=True,
                        )
                        nc.tensor.matmul(
                            sm_ps[o : o + 64, :], lhsT=ones_kv,
                            rhs=eT[:, h, :], start=True, stop=True,
                        )
                    rs = rsp.tile([128, 512], F32, tag="rs", name="rs")
                    nc.vector.reciprocal_approx_fast(rs, sm_ps)
                    nc.vector.tensor_tensor(oT_sb[:, t, :], pv_ps, rs, ALU.mult)
                oT_pending[bi] = oT_sb

                stageA_q_evac(bi + 1, [0, 1])
                stageA_q_mm(bi + 1, [2, 3])
                stageA_q_evac(bi + 1, [2, 3])

                # out-projection of the PREVIOUS block (its mults are long
                # done -> no PE stall)
                emit_out(bi - 1)

            emit_out(N_BLOCKS - 1)

    nc.compile()
    return nc


_BUILD_CACHE = {}


def _get_nc():
    if "nc" not in _BUILD_CACHE:
        _BUILD_CACHE["nc"] = build_nc()
    return _BUILD_CACHE["nc"]


def _in_maps(latent, context, wq, bq, wk, bk, wv, bv, wo, bo):
    import ml_dtypes

    bf = ml_dtypes.bfloat16
    f = lambda a: np.ascontiguousarray(np.asarray(a), dtype=np.float32)
    fb = lambda a: np.ascontiguousarray(
        np.asarray(a, dtype=np.float32).astype(bf)
    )
    shared = {
        "wq": fb(wq), "bq": f(bq), "wk": fb(wk), "bk": f(bk),
        "wv": fb(wv), "bv": fb(bv), "wo": fb(wo), "bo": fb(bo),
    }
    maps = []
    for b in range(N_CORES):
        m = dict(shared)
        m["latent"] = fb(latent[b])
        m["context"] = fb(context[b])
        maps.append(m)
    return maps


def run_on_hw(inputs, trace=False, **kw):
    nc = _get_nc()
    maps = _in_maps(**inputs)
    res = run_bass_kernel_spmd(nc, maps, list(range(N_CORES)), trace=trace, **kw)
    out = np.stack([res.results[b]["out"] for b in range(N_CORES)], axis=0)
    return out, res


def kernel(latent, context, wq, bq, wk, bk, wv, bv, wo, bo):
    out, _ = run_on_hw(dict(
        latent=latent, context=context, wq=wq, bq=bq, wk=wk, bk=bk,
        wv=wv, bv=bv, wo=wo, bo=bo,
    ))
    return out


# revision 40
# speedup vs baseline: 1.1230x; 1.0036x over previous
"""Trainium2 Bass kernel for CrossAttention.

Problem shape (hardcoded):
  latent  [8, 4096, 512], context [8, 77, 768]
  wq [512,512], wk/wv [768,512], wo [512,512], biases [512]
  out = softmax((latent@wq+bq)(context@wk+bk)^T / 8) @ (context@wv+bv) @ wo + bo

Sharding: data-parallel over batch — core b handles batch element b.

Perf design (v9), from trace analysis of earlier versions:
  - v1 was LDWEIGHTS-bound (~2400 small matmuls with 64/77-col weights ->
    no fast-weight-load, PE weight port saturated at ~110ns/MM).
  - Restructured to N=512 streaming matmuls with 128-col bf16 weights:
    steady-state MM pitch hits the theoretical 216ns.
  - Software pipelining for the in-order PE queue: stage A (transposes +
    Q-proj) of block b+1 is emitted inside stage B of block b, and the
    out-projection of block b is DEFERRED into iteration b+1 so the PE
    chews on it while block b+1's softmax/PV chain waits on ACT/DVE.
  - Inputs arrive pre-cast to bf16 (host-side dtype choice, numerically
    identical to an on-device cast); x rides the fast SWDGE queue, stores
    are spread across the sync/scalar/gpsimd DMA queues.

Dataflow per core (one batch element), bf16 matmul operands:
  prep: cT = ctx^T; kT2 [128,8,77] head-parity layout (head h in rows
        (h%2)*64..+64, other half zero -> one matmul against the full
        128-row head-pair qT tile selects head h); v_hm [128,8,64] V per
        head kv-major; ones_kv (softmax-sum lhsT); bo_bc [128,512].
  per 512-row block:
    xT = x^T (16 PE transposes), qT = wq^T xT + bq  [128,4,512] bf16
    sT_h = kT2_h^T qT_pair -> [77,512] psum; eT_h = exp(sT_h) (ACT)
    pv/sums per head PAIR stacked [even head rows 0:64 | odd 64:128] ->
        full-tile reciprocal_approx_fast + multiply on DVE (all
        base-partition-0: custom DVE ops mishandle base-64 inputs)
    out = oT^T wo (+bo via DVE add) -> DMA (deferred one iteration)
"""

import os
import sys
from contextlib import ExitStack

import numpy as np

for _p in ("/opt/trn_rl_repo",):
    if _p not in sys.path and os.path.isdir(_p):
        sys.path.insert(0, _p)

import concourse.bass as bass  # noqa: E402
import concourse.tile as tile  # noqa: E402
from concourse import bacc, mybir  # noqa: E402
from concourse.bass_utils import run_bass_kernel_spmd  # noqa: E402
from concourse.masks import make_identity  # noqa: E402

N_CORES = 8
MM_DT = "bf16"  # informational (test.py prints it)
SQ, D, DC, SKV, H, DH = 4096, 512, 768, 77, 8, 64
F32 = mybir.dt.float32
BF16 = mybir.dt.bfloat16
AF = mybir.ActivationFunctionType
ALU = mybir.AluOpType

N_BLOCKS = SQ // 512  # 8 blocks of 512 query rows


def build_nc():
    nc = bacc.Bacc("TRN2", target_bir_lowering=False, debug=False)

    lat = nc.dram_tensor("latent", [SQ, D], BF16, kind="ExternalInput").ap()
    ctx_d = nc.dram_tensor("context", [SKV, DC], BF16, kind="ExternalInput").ap()
    wq = nc.dram_tensor("wq", [D, D], BF16, kind="ExternalInput").ap()
    bq = nc.dram_tensor("bq", [D], F32, kind="ExternalInput").ap()
    wk = nc.dram_tensor("wk", [DC, D], BF16, kind="ExternalInput").ap()
    bk = nc.dram_tensor("bk", [D], F32, kind="ExternalInput").ap()
    wv = nc.dram_tensor("wv", [DC, D], BF16, kind="ExternalInput").ap()
    bv = nc.dram_tensor("bv", [D], BF16, kind="ExternalInput").ap()
    wo = nc.dram_tensor("wo", [D, D], BF16, kind="ExternalInput").ap()
    bo = nc.dram_tensor("bo", [D], BF16, kind="ExternalInput").ap()
    out_d = nc.dram_tensor("out", [SQ, D], F32, kind="ExternalOutput").ap()

    with tile.TileContext(nc) as tc:
        with ExitStack() as stk:
            consts = stk.enter_context(tc.tile_pool(name="consts", bufs=1))
            xpool = stk.enter_context(tc.tile_pool(name="x", bufs=3))
            xtp = stk.enter_context(tc.tile_pool(name="xt", bufs=2))
            qtp = stk.enter_context(tc.tile_pool(name="qt", bufs=2))
            otp = stk.enter_context(tc.tile_pool(name="ot", bufs=2))
            rsp = stk.enter_context(tc.tile_pool(name="rs", bufs=2))
            outp = stk.enter_context(tc.tile_pool(name="outp", bufs=3))
            # PSUM: 8 banks. trq: transposes + q-proj (2); spv: scores +
            # pv/sums (4); op: out-proj (2).
            trq = stk.enter_context(tc.tile_pool(name="trq", bufs=2, space="PSUM"))
            spv = stk.enter_context(tc.tile_pool(name="spv", bufs=4, space="PSUM"))
            op = stk.enter_context(tc.tile_pool(name="op", bufs=2, space="PSUM"))

            # ---------- constants the PE needs first (emitted before any
            # DMA so nothing queues behind SWDGE transfers) ----------
            ident = consts.tile([128, 128], BF16, name="ident")
            make_identity(nc, ident)
            e0 = consts.tile([128, 128], BF16, name="e0")
            nc.vector.memset(e0, 0.0)
            nc.vector.memset(e0[0:1, :], 1.0)
            eT_bufs = []
            for i in range(2):
                eT = consts.tile([128, H, 512], BF16, name=f"eT{i}")
                nc.vector.memset(eT[64:128, :, :], 0.0)
                eT_bufs.append(eT)

            # ---------- loads: x + bulk weights on SWDGE (fast), wq/ctx on
            # the HW queues so block 0 isn't gated on the SWDGE backlog ----
            x_tiles = {}

            def load_x(bi, split=False):
                if bi >= N_BLOCKS:
                    return
                t = xpool.tile([128, 4, D], BF16, tag="x", name=f"x{bi}")
                if split:
                    # per-g-chunk DMAs: the first transposes start as soon
                    # as g0 lands instead of waiting for the full 512 rows
                    for g in range(4):
                        r = bi * 512 + g * 128
                        nc.gpsimd.dma_start(t[:, g, :], lat[r : r + 128, :])
                else:
                    nc.gpsimd.dma_start(
                        t, lat[bi * 512 : (bi + 1) * 512, :].rearrange(
                            "(g p) d -> p g d", p=128
                        )
                    )
                x_tiles[bi] = t

            def loadw(eng, ap_in, shape, name):
                t = consts.tile(shape, BF16, name=name)
                eng.dma_start(t, ap_in)
                return t

            load_x(0, split=True)
            wq_sb = loadw(nc.sync, wq.rearrange("(t p) d -> p t d", p=128),
                          [128, 4, D], "wq_sb")
            ctx_sb = consts.tile([128, DC], BF16, name="ctx_sb")
            nc.vector.memset(ctx_sb, 0.0)
            nc.scalar.dma_start(ctx_sb[:SKV, :], ctx_d)
            # wk on the otherwise-idle scalar queue; wv ahead of x1 on
            # SWDGE — so the K/V prep isn't gated on the x-load backlog
            wk_r = wk.rearrange("(t p) d -> p t d", p=128)
            wk_sb = consts.tile([128, 6, D], BF16, name="wk_sb")
            nc.scalar.dma_start(wk_sb[:, 0:3, :], wk_r[:, 0:3, :])
            nc.scalar.dma_start(wk_sb[:, 3:6, :], wk_r[:, 3:6, :])
            wv_r = wv.rearrange("(t p) d -> p t d", p=128)
            wv_sb = consts.tile([128, 6, D], BF16, name="wv_sb")
            nc.gpsimd.dma_start(wv_sb[:, 0:3, :], wv_r[:, 0:3, :])
            nc.gpsimd.dma_start(wv_sb[:, 3:6, :], wv_r[:, 3:6, :])
            load_x(1)
            wo_sb = loadw(nc.gpsimd, wo.rearrange("(t p) d -> p t d", p=128),
                          [128, 4, D], "wo_sb")

            bq_sb = consts.tile([128, 4], F32, name="bq_sb")
            nc.sync.dma_start(bq_sb, bq.rearrange("(t p) -> p t", p=128))
            bk_sb = consts.tile([128, 4], F32, name="bk_sb")
            nc.sync.dma_start(bk_sb, bk.rearrange("(t p) -> p t", p=128))
            bk_s = consts.tile([128, 4], F32, name="bk_s")
            nc.vector.tensor_scalar_mul(bk_s, bk_sb, 0.125)

            bv_pad = consts.tile([128, D], BF16, name="bv_pad")
            nc.vector.memset(bv_pad, 0.0)
            nc.sync.dma_start(bv_pad[0:1, :], bv.rearrange("(o d) -> o d", o=1))
            bo_pad = consts.tile([128, D], BF16, name="bo_pad")
            nc.vector.memset(bo_pad, 0.0)
            nc.sync.dma_start(bo_pad[0:1, :], bo.rearrange("(o d) -> o d", o=1))

            # ---------- stage A pieces (emitted fine-grained) ----------
            xT_tiles, qT_tiles = {}, {}

            def stageA_tr(bi):
                """x^T via 16 PE transposes -> xT [128, 4(dchunk), 512]."""
                if bi >= N_BLOCKS:
                    return
                x_sb = x_tiles.pop(bi)
                xT_sb = xtp.tile([128, 4, 512], BF16, tag="xT", name="xT_sb")
                for g in range(4):
                    tr_ps = trq.tile([128, 4, 128], BF16, tag="trq", name="tr_ps")
                    for et in range(4):
                        nc.tensor.transpose(
                            tr_ps[:, et, :],
                            x_sb[:, g, et * 128 : (et + 1) * 128],
                            ident,
                        )
                    nc.vector.tensor_copy(
                        xT_sb[:, :, g * 128 : (g + 1) * 128], tr_ps
                    )
                xT_tiles[bi] = xT_sb

            q_ps_pend = {}

            def stageA_q_mm(bi, ms):
                """Q-proj matmuls only (evac emitted separately so block
                bi-1's exps stay ahead of these in the ACT FIFO)."""
                if bi >= N_BLOCKS:
                    return
                if bi not in qT_tiles:
                    qT_tiles[bi] = qtp.tile([128, 4, 512], BF16, tag="qT", name="qT_sb")
                xT_sb = xT_tiles[bi]
                for m in ms:
                    q_ps = trq.tile([128, 512], F32, tag="trq", name="q_ps")
                    for k in range(4):
                        nc.tensor.matmul(
                            q_ps,
                            lhsT=wq_sb[:, k, m * 128 : (m + 1) * 128],
                            rhs=xT_sb[:, k, :],
                            start=(k == 0),
                            stop=(k == 3),
                        )
                    q_ps_pend[(bi, m)] = q_ps
                if ms[-1] == 3:
                    xT_tiles.pop(bi)

            def stageA_q_evac(bi, ms):
                if bi >= N_BLOCKS:
                    return
                qT_sb = qT_tiles[bi]
                for m in ms:
                    nc.scalar.activation(
                        qT_sb[:, m, :], q_ps_pend.pop((bi, m)), AF.Identity,
                        bias=bq_sb[:, m : m + 1],
                    )

            def stageA_q(bi, ms):
                stageA_q_mm(bi, ms)
                stageA_q_evac(bi, ms)

            # deferred out-projection: emitted one iteration later so the
            # PE fills the tail of each block's DVE normalize chain
            oT_pending = {}

            def emit_out(bi):
                if bi < 0:
                    return
                oT_sb = oT_pending.pop(bi)
                store_eng = [nc.sync, nc.scalar, nc.gpsimd, nc.gpsimd]
                for r in range(4):
                    o_ps = op.tile([128, 512], F32, tag="op", name="o_ps")
                    for t in range(4):
                        nc.tensor.matmul(
                            o_ps,
                            lhsT=oT_sb[:, t, r * 128 : (r + 1) * 128],
                            rhs=wo_sb[:, t, :],
                            start=(t == 0),
                            stop=(t == 3),
                        )
                    out_sb = outp.tile([128, D], F32, tag="out", name="out_sb")
                    nc.vector.tensor_tensor(out_sb, o_ps, bo_bc, ALU.add)
                    rr = bi * 512 + r * 128
                    store_eng[r].dma_start(out_d[rr : rr + 128, :], out_sb)

            # ---------- block 0 stage A before K/V prep (HAM warmup; x0/wq
            # arrive before SWDGE finishes wk/wv) ----------
            stageA_tr(0)
            stageA_q(0, [0, 1, 2, 3])

            # ---------- K/V prep ----------
            cT_sb = consts.tile([128, 6, SKV], BF16, name="cT_sb")
            for g in range(2):
                cT_ps = trq.tile([128, 3, 128], BF16, tag="trq", name="cT_ps")
                for t3 in range(3):
                    t = g * 3 + t3
                    nc.tensor.transpose(
                        cT_ps[:, t3, :], ctx_sb[:, t * 128 : (t + 1) * 128], ident
                    )
                nc.vector.tensor_copy(cT_sb[:, 3 * g : 3 * g + 3, :], cT_ps[:, :, :SKV])

            kT2 = consts.tile([128, H, SKV], BF16, name="kT2")
            nc.vector.memset(kT2, 0.0)
            for t in range(4):
                kT_ps = spv.tile([128, SKV], F32, tag="spv", name="kT_ps")
                for ct in range(6):
                    nc.tensor.matmul(
                        kT_ps,
                        lhsT=wk_sb[:, ct, t * 128 : (t + 1) * 128],
                        rhs=cT_sb[:, ct, :],
                        start=(ct == 0),
                        stop=(ct == 5),
                    )
                nc.scalar.activation(
                    kT2[0:64, 2 * t, :], kT_ps[0:64, :], AF.Identity,
                    bias=bk_s[0:64, t : t + 1], scale=0.125,
                )
                nc.scalar.activation(
                    kT2[64:128, 2 * t + 1, :], kT_ps[64:128, :], AF.Identity,
                    bias=bk_s[64:128, t : t + 1], scale=0.125,
                )

            v_ps = spv.tile([SKV, D], F32, tag="spv", name="v_ps")
            for ct in range(6):
                nc.tensor.matmul(
                    v_ps, lhsT=cT_sb[:, ct, :], rhs=wv_sb[:, ct, :],
                    start=(ct == 0), stop=False,
                )
            nc.tensor.matmul(
                v_ps, lhsT=e0[:, :SKV], rhs=bv_pad, start=False, stop=True
            )
            v_hm = consts.tile([128, H, 64], BF16, name="v_hm")
            nc.vector.memset(v_hm, 0.0)
            for h in range(H):
                nc.vector.tensor_copy(
                    v_hm[0:SKV, h, :], v_ps[:, h * 64 : (h + 1) * 64]
                )
            ones_kv = consts.tile([128, 64], BF16, name="ones_kv")
            nc.vector.memset(ones_kv, 0.0)
            nc.vector.memset(ones_kv[0:64, :], 1.0)
            nc.vector.memset(ones_kv[64:SKV, :], 1.0)

            # bo broadcast to all 128 partitions via one rank-1 matmul
            bo_ps = op.tile([128, 512], F32, tag="op", name="bo_ps")
            nc.tensor.matmul(bo_ps, lhsT=e0, rhs=bo_pad, start=True, stop=True)
            bo_bc = consts.tile([128, D], F32, name="bo_bc")
            nc.vector.tensor_copy(bo_bc, bo_ps)

            # ---------- main loop ----------
            for bi in range(N_BLOCKS):
                eT = eT_bufs[bi % 2]
                qT_sb = qT_tiles.pop(bi)

                load_x(bi + 2)

                def sT(h):
                    s_ps = spv.tile([SKV, 512], F32, tag="spv", name="s_ps")
                    nc.tensor.matmul(
                        s_ps, lhsT=kT2[:, h, :], rhs=qT_sb[:, h // 2, :],
                        start=True, stop=True,
                    )
                    nc.scalar.activation(eT[0:SKV, h, :], s_ps, AF.Exp)

                # scores interleaved with block bi+1's transposes/Q-proj
                # matmuls; the Q-proj ACT evacs are deferred until after all
                # 8 exps so the ACT FIFO serves the softmax chain first
                sT(0); sT(1); sT(2); sT(3)
                stageA_tr(bi + 1)
                stageA_q_mm(bi + 1, [0, 1])
                sT(4); sT(5); sT(6); sT(7)

                # PV + sums per head pair, then full-tile recip * mult
                oT_sb = otp.tile([128, 4, 512], BF16, tag="oT", name="oT_sb")
                for t in range(4):
                    pv_ps = spv.tile([128, 512], F32, tag="spv", name="pv_ps")
                    sm_ps = spv.tile([128, 512], F32, tag="spv", name="sm_ps")
                    for hh in range(2):
                        h = 2 * t + hh
                        o = hh * 64
                        nc.tensor.matmul(
                            pv_ps[o : o + 64, :], lhsT=v_hm[:, h, :],
                            rhs=eT[:, h, :], start=True, stop=True,
                        )
                        nc.tensor.matmul(
                            sm_ps[o : o + 64, :], lhsT=ones_kv,
                            rhs=eT[:, h, :], start=True, stop=True,
                        )
                    rs = rsp.tile([128, 512], F32, tag="rs", name="rs")
                    nc.vector.reciprocal_approx_fast(rs, sm_ps)
                    nc.vector.tensor_tensor(oT_sb[:, t, :], pv_ps, rs, ALU.mult)
                oT_pending[bi] = oT_sb

                stageA_q_evac(bi + 1, [0, 1])
                stageA_q_mm(bi + 1, [2, 3])
                stageA_q_evac(bi + 1, [2, 3])

                # out-projection of the PREVIOUS block (its mults are long
                # done -> no PE stall)
                emit_out(bi - 1)

            emit_out(N_BLOCKS - 1)

    nc.compile()
    return nc


_BUILD_CACHE = {}


def _get_nc():
    if "nc" not in _BUILD_CACHE:
        _BUILD_CACHE["nc"] = build_nc()
    return _BUILD_CACHE["nc"]


def _in_maps(latent, context, wq, bq, wk, bk, wv, bv, wo, bo):
    import ml_dtypes

    bf = ml_dtypes.bfloat16
    f = lambda a: np.ascontiguousarray(np.asarray(a), dtype=np.float32)
    fb = lambda a: np.ascontiguousarray(
        np.asarray(a, dtype=np.float32).astype(bf)
    )
    shared = {
        "wq": fb(wq), "bq": f(bq), "wk": fb(wk), "bk": f(bk),
        "wv": fb(wv), "bv": fb(bv), "wo": fb(wo), "bo": fb(bo),
    }
    maps = []
    for b in range(N_CORES):
        m = dict(shared)
        m["latent"] = fb(latent[b])
        m["context"] = fb(context[b])
        maps.append(m)
    return maps


def run_on_hw(inputs, trace=False, **kw):
    nc = _get_nc()
    maps = _in_maps(**inputs)
    res = run_bass_kernel_spmd(nc, maps, list(range(N_CORES)), trace=trace, **kw)
    out = np.stack([res.results[b]["out"] for b in range(N_CORES)], axis=0)
    return out, res


def kernel(latent, context, wq, bq, wk, bk, wv, bv, wo, bo):
    out, _ = run_on_hw(dict(
        latent=latent, context=context, wq=wq, bq=bq, wk=wk, bk=bk,
        wv=wv, bv=bv, wo=wo, bo=bo,
    ))
    return out


# revision 43
# speedup vs baseline: 1.1570x; 1.0303x over previous
"""Trainium2 Bass kernel for CrossAttention.

Problem shape (hardcoded):
  latent  [8, 4096, 512], context [8, 77, 768]
  wq [512,512], wk/wv [768,512], wo [512,512], biases [512]
  out = softmax((latent@wq+bq)(context@wk+bk)^T / 8) @ (context@wv+bv) @ wo + bo

Sharding: data-parallel over batch — core b handles batch element b.

Perf design (v9), from trace analysis of earlier versions:
  - v1 was LDWEIGHTS-bound (~2400 small matmuls with 64/77-col weights ->
    no fast-weight-load, PE weight port saturated at ~110ns/MM).
  - Restructured to N=512 streaming matmuls with 128-col bf16 weights:
    steady-state MM pitch hits the theoretical 216ns.
  - Software pipelining for the in-order PE queue: stage A (transposes +
    Q-proj) of block b+1 is emitted inside stage B of block b, and the
    out-projection of block b is DEFERRED into iteration b+1 so the PE
    chews on it while block b+1's softmax/PV chain waits on ACT/DVE.
  - Inputs arrive pre-cast to bf16 (host-side dtype choice, numerically
    identical to an on-device cast); x rides the fast SWDGE queue, stores
    are spread across the sync/scalar/gpsimd DMA queues.

Dataflow per core (one batch element), bf16 matmul operands:
  prep: cT = ctx^T; kT2 [128,8,77] head-parity layout (head h in rows
        (h%2)*64..+64, other half zero -> one matmul against the full
        128-row head-pair qT tile selects head h); v_hm [128,8,64] V per
        head kv-major; ones_kv (softmax-sum lhsT); bo_bc [128,512].
  per 512-row block:
    xT = x^T (16 PE transposes), qT = wq^T xT + bq  [128,4,512] bf16
    sT_h = kT2_h^T qT_pair -> [77,512] psum; eT_h = exp(sT_h) (ACT)
    pv/sums per head PAIR stacked [even head rows 0:64 | odd 64:128] ->
        full-tile reciprocal_approx_fast + multiply on DVE (all
        base-partition-0: custom DVE ops mishandle base-64 inputs)
    out = oT^T wo (+bo via DVE add) -> DMA (deferred one iteration)
"""

import os
import sys
from contextlib import ExitStack

import numpy as np

for _p in ("/opt/trn_rl_repo",):
    if _p not in sys.path and os.path.isdir(_p):
        sys.path.insert(0, _p)

import concourse.bass as bass  # noqa: E402
import concourse.tile as tile  # noqa: E402
from concourse import bacc, mybir  # noqa: E402
from concourse.bass_utils import run_bass_kernel_spmd  # noqa: E402
from concourse.masks import make_identity  # noqa: E402

N_CORES = 8
MM_DT = "bf16"  # informational (test.py prints it)
SQ, D, DC, SKV, H, DH = 4096, 512, 768, 77, 8, 64
F32 = mybir.dt.float32
BF16 = mybir.dt.bfloat16
AF = mybir.ActivationFunctionType
ALU = mybir.AluOpType

N_BLOCKS = SQ // 512  # 8 blocks of 512 query rows


def build_nc():
    nc = bacc.Bacc("TRN2", target_bir_lowering=False, debug=False)

    lat = nc.dram_tensor("latent", [SQ, D], BF16, kind="ExternalInput").ap()
    ctx_d = nc.dram_tensor("context", [SKV, DC], BF16, kind="ExternalInput").ap()
    wq = nc.dram_tensor("wq", [D, D], BF16, kind="ExternalInput").ap()
    bq = nc.dram_tensor("bq", [D], F32, kind="ExternalInput").ap()
    wk = nc.dram_tensor("wk", [DC, D], BF16, kind="ExternalInput").ap()
    bk = nc.dram_tensor("bk", [D], F32, kind="ExternalInput").ap()
    wv = nc.dram_tensor("wv", [DC, D], BF16, kind="ExternalInput").ap()
    bv = nc.dram_tensor("bv", [D], BF16, kind="ExternalInput").ap()
    wo = nc.dram_tensor("wo", [D, D], BF16, kind="ExternalInput").ap()
    bo = nc.dram_tensor("bo", [D], BF16, kind="ExternalInput").ap()
    out_d = nc.dram_tensor("out", [SQ, D], F32, kind="ExternalOutput").ap()

    with tile.TileContext(nc) as tc:
        with ExitStack() as stk:
            consts = stk.enter_context(tc.tile_pool(name="consts", bufs=1))
            xpool = stk.enter_context(tc.tile_pool(name="x", bufs=3))
            xtp = stk.enter_context(tc.tile_pool(name="xt", bufs=2))
            qtp = stk.enter_context(tc.tile_pool(name="qt", bufs=2))
            otp = stk.enter_context(tc.tile_pool(name="ot", bufs=2))
            rsp = stk.enter_context(tc.tile_pool(name="rs", bufs=2))
            outp = stk.enter_context(tc.tile_pool(name="outp", bufs=3))
            # PSUM: 8 banks. trq: transposes + q-proj (2); spv: scores +
            # pv/sums (4); op: out-proj (2).
            trq = stk.enter_context(tc.tile_pool(name="trq", bufs=2, space="PSUM"))
            spv = stk.enter_context(tc.tile_pool(name="spv", bufs=4, space="PSUM"))
            op = stk.enter_context(tc.tile_pool(name="op", bufs=2, space="PSUM"))

            # ---------- constants the PE needs first (emitted before any
            # DMA so nothing queues behind SWDGE transfers) ----------
            ident = consts.tile([128, 128], BF16, name="ident")
            make_identity(nc, ident)
            e0 = consts.tile([128, 128], BF16, name="e0")
            nc.vector.memset(e0, 0.0)
            nc.vector.memset(e0[0:1, :], 1.0)
            eT_bufs = []
            for i in range(2):
                eT = consts.tile([128, H, 512], BF16, name=f"eT{i}")
                nc.vector.memset(eT[64:128, :, :], 0.0)
                eT_bufs.append(eT)

            # ---------- loads: x + bulk weights on SWDGE (fast), wq/ctx on
            # the HW queues so block 0 isn't gated on the SWDGE backlog ----
            x_tiles = {}

            def load_x(bi, split=False):
                if bi >= N_BLOCKS:
                    return
                t = xpool.tile([128, 4, D], BF16, tag="x", name=f"x{bi}")
                if split:
                    # per-g-chunk DMAs: the first transposes start as soon
                    # as g0 lands instead of waiting for the full 512 rows
                    for g in range(4):
                        r = bi * 512 + g * 128
                        nc.gpsimd.dma_start(t[:, g, :], lat[r : r + 128, :])
                else:
                    nc.gpsimd.dma_start(
                        t, lat[bi * 512 : (bi + 1) * 512, :].rearrange(
                            "(g p) d -> p g d", p=128
                        )
                    )
                x_tiles[bi] = t

            def loadw(eng, ap_in, shape, name):
                t = consts.tile(shape, BF16, name=name)
                eng.dma_start(t, ap_in)
                return t

            load_x(0, split=True)
            wq_sb = loadw(nc.sync, wq.rearrange("(t p) d -> p t d", p=128),
                          [128, 4, D], "wq_sb")
            ctx_sb = consts.tile([128, DC], BF16, name="ctx_sb")
            nc.vector.memset(ctx_sb, 0.0)
            nc.scalar.dma_start(ctx_sb[:SKV, :], ctx_d)
            # wk on the otherwise-idle scalar queue; wv ahead of x1 on
            # SWDGE — so the K/V prep isn't gated on the x-load backlog
            wk_r = wk.rearrange("(t p) d -> p t d", p=128)
            wk_sb = consts.tile([128, 6, D], BF16, name="wk_sb")
            nc.scalar.dma_start(wk_sb[:, 0:3, :], wk_r[:, 0:3, :])
            nc.scalar.dma_start(wk_sb[:, 3:6, :], wk_r[:, 3:6, :])
            wv_r = wv.rearrange("(t p) d -> p t d", p=128)
            wv_sb = consts.tile([128, 6, D], BF16, name="wv_sb")
            nc.gpsimd.dma_start(wv_sb[:, 0:3, :], wv_r[:, 0:3, :])
            nc.gpsimd.dma_start(wv_sb[:, 3:6, :], wv_r[:, 3:6, :])
            load_x(1)
            wo_sb = loadw(nc.gpsimd, wo.rearrange("(t p) d -> p t d", p=128),
                          [128, 4, D], "wo_sb")

            bq_sb = consts.tile([128, 4], F32, name="bq_sb")
            nc.sync.dma_start(bq_sb, bq.rearrange("(t p) -> p t", p=128))
            bk_sb = consts.tile([128, 4], F32, name="bk_sb")
            nc.sync.dma_start(bk_sb, bk.rearrange("(t p) -> p t", p=128))
            bk_s = consts.tile([128, 4], F32, name="bk_s")
            nc.vector.tensor_scalar_mul(bk_s, bk_sb, 0.125)

            bv_pad = consts.tile([128, D], BF16, name="bv_pad")
            nc.vector.memset(bv_pad, 0.0)
            nc.sync.dma_start(bv_pad[0:1, :], bv.rearrange("(o d) -> o d", o=1))
            bo_pad = consts.tile([128, D], BF16, name="bo_pad")
            nc.vector.memset(bo_pad, 0.0)
            nc.sync.dma_start(bo_pad[0:1, :], bo.rearrange("(o d) -> o d", o=1))

            # ---------- stage A pieces (emitted fine-grained) ----------
            xT_tiles, qT_tiles = {}, {}

            def stageA_tr(bi):
                """x^T via 16 PE transposes -> xT [128, 4(dchunk), 512]."""
                if bi >= N_BLOCKS:
                    return
                x_sb = x_tiles.pop(bi)
                xT_sb = xtp.tile([128, 4, 512], BF16, tag="xT", name="xT_sb")
                for g in range(4):
                    tr_ps = trq.tile([128, 4, 128], BF16, tag="trq", name="tr_ps")
                    for et in range(4):
                        nc.tensor.transpose(
                            tr_ps[:, et, :],
                            x_sb[:, g, et * 128 : (et + 1) * 128],
                            ident,
                        )
                    nc.vector.tensor_copy(
                        xT_sb[:, :, g * 128 : (g + 1) * 128], tr_ps
                    )
                xT_tiles[bi] = xT_sb

            q_ps_pend = {}

            def stageA_q_mm(bi, ms):
                """Q-proj matmuls only (evac emitted separately so block
                bi-1's exps stay ahead of these in the ACT FIFO)."""
                if bi >= N_BLOCKS:
                    return
                if bi not in qT_tiles:
                    qT_tiles[bi] = qtp.tile([128, 4, 512], BF16, tag="qT", name="qT_sb")
                xT_sb = xT_tiles[bi]
                for m in ms:
                    q_ps = trq.tile([128, 512], F32, tag="trq", name="q_ps")
                    for k in range(4):
                        nc.tensor.matmul(
                            q_ps,
                            lhsT=wq_sb[:, k, m * 128 : (m + 1) * 128],
                            rhs=xT_sb[:, k, :],
                            start=(k == 0),
                            stop=(k == 3),
                        )
                    q_ps_pend[(bi, m)] = q_ps
                if ms[-1] == 3:
                    xT_tiles.pop(bi)

            def stageA_q_evac(bi, ms):
                if bi >= N_BLOCKS:
                    return
                qT_sb = qT_tiles[bi]
                for m in ms:
                    nc.scalar.activation(
                        qT_sb[:, m, :], q_ps_pend.pop((bi, m)), AF.Identity,
                        bias=bq_sb[:, m : m + 1],
                    )

            def stageA_q(bi, ms):
                stageA_q_mm(bi, ms)
                stageA_q_evac(bi, ms)

            # deferred out-projection: emitted one iteration later so the
            # PE fills the tail of each block's DVE normalize chain
            oT_pending = {}

            def emit_out(bi, interleave=False):
                if bi < 0:
                    return
                oT_sb = oT_pending.pop(bi)
                store_eng = [nc.sync, nc.scalar, nc.gpsimd, nc.gpsimd]

                def finish(r, o_ps):
                    out_sb = outp.tile([128, D], F32, tag="out", name="out_sb")
                    nc.vector.tensor_tensor(out_sb, o_ps, bo_bc, ALU.add)
                    rr = bi * 512 + r * 128
                    store_eng[r].dma_start(out_d[rr : rr + 128, :], out_sb)

                if not interleave:
                    for r in range(4):
                        o_ps = op.tile([128, 512], F32, tag="op", name="o_ps")
                        for t in range(4):
                            nc.tensor.matmul(
                                o_ps,
                                lhsT=oT_sb[:, t, r * 128 : (r + 1) * 128],
                                rhs=wo_sb[:, t, :],
                                start=(t == 0),
                                stop=(t == 3),
                            )
                        finish(r, o_ps)
                    return
                # final block: interleave r-pairs t-outer so the early t
                # matmuls only wait on the early normalize multiplies
                for rp in (0, 2):
                    ps = {
                        r: op.tile([128, 512], F32, tag="op", name="o_ps")
                        for r in (rp, rp + 1)
                    }
                    for t in range(4):
                        for r in (rp, rp + 1):
                            nc.tensor.matmul(
                                ps[r],
                                lhsT=oT_sb[:, t, r * 128 : (r + 1) * 128],
                                rhs=wo_sb[:, t, :],
                                start=(t == 0),
                                stop=(t == 3),
                            )
                    for r in (rp, rp + 1):
                        finish(r, ps[r])

            # ---------- block 0 stage A before K/V prep (HAM warmup; x0/wq
            # arrive before SWDGE finishes wk/wv) ----------
            stageA_tr(0)
            stageA_q(0, [0, 1, 2, 3])

            # ---------- K/V prep ----------
            cT_sb = consts.tile([128, 6, SKV], BF16, name="cT_sb")
            for g in range(2):
                cT_ps = trq.tile([128, 3, 128], BF16, tag="trq", name="cT_ps")
                for t3 in range(3):
                    t = g * 3 + t3
                    nc.tensor.transpose(
                        cT_ps[:, t3, :], ctx_sb[:, t * 128 : (t + 1) * 128], ident
                    )
                nc.vector.tensor_copy(cT_sb[:, 3 * g : 3 * g + 3, :], cT_ps[:, :, :SKV])

            # kT accumulation ct-outer: the first 12 matmuls only need the
            # first wk half; the V matmuls (needing wv, which lands early on
            # SWDGE) slot between the halves.
            kT2 = consts.tile([128, H, SKV], BF16, name="kT2")
            nc.vector.memset(kT2, 0.0)
            kT_pss = [
                spv.tile([128, SKV], F32, tag="spv", name=f"kT_ps{t}")
                for t in range(4)
            ]
            for ct in range(3):
                for t in range(4):
                    nc.tensor.matmul(
                        kT_pss[t],
                        lhsT=wk_sb[:, ct, t * 128 : (t + 1) * 128],
                        rhs=cT_sb[:, ct, :],
                        start=(ct == 0),
                        stop=False,
                    )

            v_ps = op.tile([SKV, D], F32, tag="op", name="v_ps")
            for ct in range(6):
                nc.tensor.matmul(
                    v_ps, lhsT=cT_sb[:, ct, :], rhs=wv_sb[:, ct, :],
                    start=(ct == 0), stop=False,
                )
            nc.tensor.matmul(
                v_ps, lhsT=e0[:, :SKV], rhs=bv_pad, start=False, stop=True
            )

            for ct in range(3, 6):
                for t in range(4):
                    nc.tensor.matmul(
                        kT_pss[t],
                        lhsT=wk_sb[:, ct, t * 128 : (t + 1) * 128],
                        rhs=cT_sb[:, ct, :],
                        start=False,
                        stop=(ct == 5),
                    )
            for t in range(4):
                nc.scalar.activation(
                    kT2[0:64, 2 * t, :], kT_pss[t][0:64, :], AF.Identity,
                    bias=bk_s[0:64, t : t + 1], scale=0.125,
                )
                nc.scalar.activation(
                    kT2[64:128, 2 * t + 1, :], kT_pss[t][64:128, :], AF.Identity,
                    bias=bk_s[64:128, t : t + 1], scale=0.125,
                )
            v_hm = consts.tile([128, H, 64], BF16, name="v_hm")
            nc.vector.memset(v_hm, 0.0)
            for h in range(H):
                nc.vector.tensor_copy(
                    v_hm[0:SKV, h, :], v_ps[:, h * 64 : (h + 1) * 64]
                )
            ones_kv = consts.tile([128, 64], BF16, name="ones_kv")
            nc.vector.memset(ones_kv, 0.0)
            nc.vector.memset(ones_kv[0:64, :], 1.0)
            nc.vector.memset(ones_kv[64:SKV, :], 1.0)

            # bo broadcast to all 128 partitions via one rank-1 matmul
            bo_ps = op.tile([128, 512], F32, tag="op", name="bo_ps")
            nc.tensor.matmul(bo_ps, lhsT=e0, rhs=bo_pad, start=True, stop=True)
            bo_bc = consts.tile([128, D], F32, name="bo_bc")
            nc.vector.tensor_copy(bo_bc, bo_ps)

            # ---------- main loop ----------
            for bi in range(N_BLOCKS):
                eT = eT_bufs[bi % 2]
                qT_sb = qT_tiles.pop(bi)

                load_x(bi + 2)

                def sT(h):
                    s_ps = spv.tile([SKV, 512], F32, tag="spv", name="s_ps")
                    nc.tensor.matmul(
                        s_ps, lhsT=kT2[:, h, :], rhs=qT_sb[:, h // 2, :],
                        start=True, stop=True,
                    )
                    nc.scalar.activation(eT[0:SKV, h, :], s_ps, AF.Exp)

                # scores interleaved with block bi+1's transposes/Q-proj
                # matmuls; the Q-proj ACT evacs are deferred until after all
                # 8 exps so the ACT FIFO serves the softmax chain first
                sT(0); sT(1); sT(2); sT(3)
                stageA_tr(bi + 1)
                stageA_q_mm(bi + 1, [0, 1])
                sT(4); sT(5); sT(6); sT(7)

                # PV + sums per head pair, then full-tile recip * mult
                oT_sb = otp.tile([128, 4, 512], BF16, tag="oT", name="oT_sb")
                for t in range(4):
                    pv_ps = spv.tile([128, 512], F32, tag="spv", name="pv_ps")
                    sm_ps = spv.tile([128, 512], F32, tag="spv", name="sm_ps")
                    for hh in range(2):
                        h = 2 * t + hh
                        o = hh * 64
                        nc.tensor.matmul(
                            pv_ps[o : o + 64, :], lhsT=v_hm[:, h, :],
                            rhs=eT[:, h, :], start=True, stop=True,
                        )
                        nc.tensor.matmul(
                            sm_ps[o : o + 64, :], lhsT=ones_kv,
                            rhs=eT[:, h, :], start=True, stop=True,
                        )
                    rs = rsp.tile([128, 512], F32, tag="rs", name="rs")
                    nc.vector.reciprocal_approx_fast(rs, sm_ps)
                    nc.vector.tensor_tensor(oT_sb[:, t, :], pv_ps, rs, ALU.mult)
                oT_pending[bi] = oT_sb

                stageA_q_evac(bi + 1, [0, 1])
                stageA_q_mm(bi + 1, [2, 3])
                stageA_q_evac(bi + 1, [2, 3])

                # out-projection of the PREVIOUS block (its mults are long
                # done -> no PE stall)
                emit_out(bi - 1)

            emit_out(N_BLOCKS - 1, interleave=True)

    nc.compile()
    return nc


_BUILD_CACHE = {}


def _get_nc():
    if "nc" not in _BUILD_CACHE:
        _BUILD_CACHE["nc"] = build_nc()
    return _BUILD_CACHE["nc"]


def _in_maps(latent, context, wq, bq, wk, bk, wv, bv, wo, bo):
    import ml_dtypes

    bf = ml_dtypes.bfloat16
    f = lambda a: np.ascontiguousarray(np.asarray(a), dtype=np.float32)
    fb = lambda a: np.ascontiguousarray(
        np.asarray(a, dtype=np.float32).astype(bf)
    )
    shared = {
        "wq": fb(wq), "bq": f(bq), "wk": fb(wk), "bk": f(bk),
        "wv": fb(wv), "bv": fb(bv), "wo": fb(wo), "bo": fb(bo),
    }
    maps = []
    for b in range(N_CORES):
        m = dict(shared)
        m["latent"] = fb(latent[b])
        m["context"] = fb(context[b])
        maps.append(m)
    return maps


def run_on_hw(inputs, trace=False, **kw):
    nc = _get_nc()
    maps = _in_maps(**inputs)
    res = run_bass_kernel_spmd(nc, maps, list(range(N_CORES)), trace=trace, **kw)
    out = np.stack([res.results[b]["out"] for b in range(N_CORES)], axis=0)
    return out, res


def kernel(latent, context, wq, bq, wk, bk, wv, bv, wo, bo):
    out, _ = run_on_hw(dict(
        latent=latent, context=context, wq=wq, bq=bq, wk=wk, bk=bk,
        wv=wv, bv=bv, wo=wo, bo=bo,
    ))
    return out
